# revision 18
# baseline (speedup 1.0000x reference)
"""Trainium2 Bass kernel for nn_EncoderTransformer_61194694033513.

Data-parallel over batch B=16 across 8 NeuronCores (2 batch elems per core).
Per core the whole forward runs out of SBUF with activations feature-major
HT[e, tok] in fp16; all matmul accumulation is fp32 in PSUM.

Key structure (v4):
- S = relu(Q K^T) computed as relu(H G H^T) with G = Wq Wk^T folded on the
  host: only one projected tensor k' = H G (no q projection); the S^T lhsT
  reads the H tiles directly.
- H ping-pongs between two tile sets per batch elem so the attention residual
  write never aliases the key operand of later S tiles.
- Two batch streams interleaved phase-by-phase; additionally all LN work is
  chunk-level software-pipelined: squares inline after each chunk's residual,
  stats matmuls+row math one chunk behind, and the broadcast/apply steps are
  queued as "fillers" popped between chunks of the next PE phase, so no
  engine FIFO ever blocks the PE.
- Attention inner loop pipelines S(j2+1) before O(j2); relu split across
  scalar+vector per 512-half.
- LN broadcasts: one gpsimd partition_broadcast per 1024-token half carrying
  [rstd|mean*rstd]; apply is (mb*g-be) on scalar + 2 DVE ops, 1024 wide.
- Readout folded into the last LN apply: pr = t2 * (g*Wout) host-fused,
  ones-matmul reduced; be2 contribution folded into b_out on the host.
"""

import sys

import numpy as np

for _p in (
    "/opt/trn_rl_repo",
    "/opt/pypackages",
    "/root/.axon_site",
    "/root/.axon_site/_ro/trn_rl_repo",
    "/root/.axon_site/_ro/pypackages",
):
    if _p not in sys.path:
        sys.path.append(_p)

import ml_dtypes  # noqa: E402,F401

import concourse.bass as bass  # noqa: E402
import concourse.bacc as bacc  # noqa: E402
import concourse.mybir as mybir  # noqa: E402
from concourse import tile  # noqa: E402
from concourse.bass_utils import run_bass_kernel_spmd  # noqa: E402

B, N, D, E, L = 16, 2048, 128, 256, 3
NCORES = 8
BL = B // NCORES  # batch elems per core
P = 128
EC = E // P  # feature-dim partition chunks (2)
CH = N // 512  # 512-wide token chunks (4)
JT = N // P  # key tiles (16)
EPS = 1e-5
F32 = mybir.dt.float32
F16 = mybir.dt.float16
NPF16 = np.float16
AF = mybir.ActivationFunctionType
OP = mybir.AluOpType

_CACHE = {}


def _build():
    from collections import deque

    nc = bacc.Bacc("TRN2", target_bir_lowering=False, debug=False, num_devices=NCORES)

    d_xsT = nc.declare_dram_parameter("xsT", [BL, P, N], F16, isOutput=False)
    d_Win = nc.declare_dram_parameter("Win", [D, E], F16, isOutput=False)
    d_W = {
        nm: nc.declare_dram_parameter(nm, [L, E, E], F16, isOutput=False)
        for nm in ("G", "Wv", "W1", "W2")
    }
    d_gwout = nc.declare_dram_parameter("gwoutT", [E, N], F16, isOutput=False)
    d_colpack = nc.declare_dram_parameter("colpack", [P, 2 + 8 * L * EC], F32, isOutput=False)
    d_bout = nc.declare_dram_parameter("b_out", [1, 1], F32, isOutput=False)
    d_out = nc.declare_dram_parameter("out", [BL, 1], F32, isOutput=True)

    with tile.TileContext(nc) as tc:
        from contextlib import ExitStack

        with ExitStack() as ctx:
            cpool = ctx.enter_context(tc.tile_pool(name="const", bufs=1))
            hpool = ctx.enter_context(tc.tile_pool(name="acts", bufs=1))
            xs_pool = ctx.enter_context(tc.tile_pool(name="xs", bufs=2))
            spool = ctx.enter_context(tc.tile_pool(name="srelu", bufs=4))
            sqpool = ctx.enter_context(tc.tile_pool(name="sqp", bufs=5))
            apool = ctx.enter_context(tc.tile_pool(name="mlpa", bufs=4))
            tpool = ctx.enter_context(tc.tile_pool(name="t1p", bufs=3))
            prpool = ctx.enter_context(tc.tile_pool(name="prp", bufs=4))
            bcpool = ctx.enter_context(tc.tile_pool(name="bc", bufs=3))
            mgpool = ctx.enter_context(tc.tile_pool(name="mg", bufs=3))
            rspool = ctx.enter_context(tc.tile_pool(name="rs", bufs=2))
            ropool = ctx.enter_context(tc.tile_pool(name="ro", bufs=2))

            PS = bass.MemorySpace.PSUM
            ps_s = ctx.enter_context(tc.tile_pool(name="ps_s", bufs=2, space=PS))
            ps_o = ctx.enter_context(tc.tile_pool(name="ps_o", bufs=2, space=PS))
            ps_mm = ctx.enter_context(tc.tile_pool(name="ps_mm", bufs=2, space=PS))

            # ---- input DMAs, ordered by first use ------------------------
            win_sb = cpool.tile([P, E], F16, name="win", tag="win")
            nc.sync.dma_start(win_sb[:], d_Win[:])
            colpack = cpool.tile([P, 2 + 8 * L * EC], F32, name="colpack", tag="colpack")
            nc.sync.dma_start(colpack[:], d_colpack[:])
            binp_sb = colpack[:, 0:EC]

            xs_tiles = [
                xs_pool.tile([P, N], F16, name=f"xst{b}", tag=f"xst{b}")
                for b in range(BL)
            ]
            for c in range(CH):
                cs = slice(c * 512, (c + 1) * 512)
                nc.sync.dma_start(xs_tiles[0][:, cs], d_xsT[0][:, cs])

            w_big = {nm: [None, None] for nm in ("G", "Wv", "W1", "W2")}
            w_sb = {nm: [None] * L for nm in ("G", "Wv", "W1", "W2")}

            def load_w(nm, ec):
                t = cpool.tile([P, L * E], F16, name=f"{nm}B{ec}", tag=f"{nm}B{ec}")
                nc.sync.dma_start(
                    t[:].rearrange("p (l e) -> p l e", l=L),
                    d_W[nm][:, ec * P : (ec + 1) * P, :].rearrange("l p e -> p l e"),
                )
                w_big[nm][ec] = t

            for ec in range(EC):
                load_w("G", ec)
            for c in range(CH):
                cs = slice(c * 512, (c + 1) * 512)
                nc.sync.dma_start(xs_tiles[1][:, cs], d_xsT[1][:, cs])
            for ec in range(EC):
                load_w("Wv", ec)
            for ec in range(EC):
                load_w("W1", ec)
            for ec in range(EC):
                load_w("W2", ec)
            for nm in w_big:
                for l in range(L):
                    w_sb[nm][l] = [
                        w_big[nm][ec][:, l * E : (l + 1) * E] for ec in range(EC)
                    ]

            def col_views(base):
                return [
                    colpack[:, 2 + base * L * EC + l * EC : 2 + base * L * EC + (l + 1) * EC]
                    for l in range(L)
                ]

            bm1_sb = col_views(0)
            bm2_sb = col_views(1)
            g1_sb = col_views(4)
            g2_sb = col_views(5)
            nbe1_sb = col_views(6)
            nbe2_sb = col_views(7)
            bout_sb = cpool.tile([1, 1], F32, name="bout", tag="bout")
            nc.sync.dma_start(bout_sb[:], d_bout[:])
            gw_sb = []
            for ec in range(EC):
                t = cpool.tile([P, N], F16, name=f"gwout{ec}", tag=f"gwout{ec}")
                nc.sync.dma_start(t[:], d_gwout[ec * P : (ec + 1) * P, :])
                gw_sb.append(t)

            ones_kb = cpool.tile([P, 1], F16, name="ones_kb", tag="ones_kb")
            nc.vector.memset(ones_kb[:], 1.0)
            eps1 = cpool.tile([1, 1], F32, name="eps1", tag="eps1")
            nc.vector.memset(eps1[:], EPS)
            # preload both scalar-engine activation table sets during DMA wait
            warm = cpool.tile([1, 1], F32, name="warm", tag="warm")
            nc.scalar.activation(warm[:], eps1[:], AF.Relu)
            nc.scalar.activation(warm[:], eps1[:], AF.Abs_reciprocal_sqrt, bias=eps1[:], scale=1.0)

            # LN rows per stream: per 1024-token half c2 a 2048-col block
            # [rstd(2c2)|rstd(2c2+1)|mrstd(2c2)|mrstd(2c2+1)]
            rowsB = [
                cpool.tile([1, 2 * N], F16, name=f"rowsB{b}", tag=f"rowsB{b}")
                for b in range(BL)
            ]

            # ---- persistent activations (fp16) ---------------------------
            HA = [[hpool.tile([P, N], F16, name=f"HA{b}{ec}", tag=f"HA{b}{ec}") for ec in range(EC)] for b in range(BL)]
            HB = [[hpool.tile([P, N], F16, name=f"HB{b}{ec}", tag=f"HB{b}{ec}") for ec in range(EC)] for b in range(BL)]
            kT = [[hpool.tile([P, N], F16, name=f"kT{b}{dc}", tag=f"kT{b}{dc}") for dc in range(EC)] for b in range(BL)]
            v_sb = [hpool.tile([P, JT * E], F16, name=f"v{b}", tag=f"v{b}") for b in range(BL)]
            ro_ps = [None] * BL
            sq_stash = {}

            # ---- filler queue -------------------------------------------
            q = deque()

            def pop_fill(k=2):
                for _ in range(k):
                    if not q:
                        return
                    q.popleft()()

            def flush_fill():
                while q:
                    q.popleft()()

            # ---- phase emitters -----------------------------------------
            def input_proj(b, X):
                xs_t = xs_tiles[b]
                for c in range(CH):
                    cs = slice(c * 512, (c + 1) * 512)
                    for ec in range(EC):
                        es = slice(ec * P, (ec + 1) * P)
                        ps = ps_mm.tile([P, 512], F32, name="psin", tag="mm")
                        nc.tensor.matmul(ps[:], win_sb[:, es], xs_t[:, cs])
                        nc.vector.tensor_scalar_add(X[ec][:, cs], ps[:], binp_sb[:, ec : ec + 1])

            def kv(b, l, X):
                # k'T first (1024-wide psum + single wide copy per dc/c-pair),
                # then the 16 v blocks; fillers drain between v groups.
                for cp in range(CH // 2):
                    for dc in range(EC):
                        ds_ = slice(dc * P, (dc + 1) * P)
                        ps = ps_s.tile([P, 1024], F32, name="psk", tag="s")
                        for ci in range(2):
                            c = 2 * cp + ci
                            cs = slice(c * 512, (c + 1) * 512)
                            for ec in range(EC):
                                nc.tensor.matmul(
                                    ps[:, ci * 512 : (ci + 1) * 512],
                                    w_sb["G"][l][ec][:, ds_],
                                    X[ec][:, cs],
                                    start=(ec == 0),
                                    stop=(ec == EC - 1),
                                )
                        dst = kT[b][dc][:, cp * 1024 : (cp + 1) * 1024]
                        if (dc + cp) % 2 == 0:
                            nc.scalar.copy(dst, ps[:])
                        else:
                            nc.vector.tensor_copy(dst, ps[:])
                for t in range(JT):
                    pool = ps_o if t % 2 == 0 else ps_mm
                    ps = pool.tile([P, E], F32, name="psv", tag="o" if t % 2 == 0 else "mm")
                    for ec in range(EC):
                        nc.tensor.matmul(
                            ps[:],
                            X[ec][:, t * P : (t + 1) * P],
                            w_sb["Wv"][l][ec][:],
                            start=(ec == 0),
                            stop=(ec == EC - 1),
                        )
                    if t % 2 == 0:
                        nc.scalar.copy(v_sb[b][:, t * E : (t + 1) * E], ps[:])
                    else:
                        nc.vector.tensor_copy(v_sb[b][:, t * E : (t + 1) * E], ps[:])
                    if t % 4 == 3:
                        pop_fill(2)

            # ---- LN pieces ----------------------------------------------
            def stats_sq(b, Y, cp):
                """Squares for chunk pair cp (1024-wide, one op per pt)."""
                cs2 = slice(cp * 1024, (cp + 1) * 1024)
                sqc = []
                for pt in range(EC):
                    sq = sqpool.tile([P, 1024], F16, name="sq", tag="sq")
                    nc.scalar.activation(sq[:], Y[pt][:, cs2], AF.Square, scale=1.0 / 64)
                    sqc.append(sq)
                sq_stash[(b, cp)] = sqc

            def stats_mm(b, Y, c):
                """Stats matmuls + row math for chunk c (emitted 1 chunk late)."""
                cs = slice(c * 512, (c + 1) * 512)
                rB = rowsB[b]
                rsl = slice(2048 * (c // 2) + 512 * (c % 2), 2048 * (c // 2) + 512 * (c % 2) + 512)
                msl = slice(2048 * (c // 2) + 1024 + 512 * (c % 2), 2048 * (c // 2) + 1024 + 512 * (c % 2) + 512)
                sqc = sq_stash[(b, c // 2)]
                if c % 2 == 1:
                    del sq_stash[(b, c // 2)]
                sqs = slice((c % 2) * 512, (c % 2) * 512 + 512)
                st_s = ps_mm.tile([1, 512], F32, name="st_s", tag="mm")
                nc.tensor.matmul(st_s[:], ones_kb[:], Y[0][:, cs], start=True, stop=False)
                nc.tensor.matmul(st_s[:], ones_kb[:], Y[1][:, cs], start=False, stop=True)
                st_q = ps_mm.tile([1, 512], F32, name="st_q", tag="mm")
                nc.tensor.matmul(st_q[:], ones_kb[:], sqc[0][:, sqs], start=True, stop=False)
                nc.tensor.matmul(st_q[:], ones_kb[:], sqc[1][:, sqs], start=False, stop=True)
                s0c = rspool.tile([1, 512], F32, name="s0c", tag="s0c")
                nc.scalar.copy(s0c[:], st_s[:])
                xrc = rspool.tile([1, 512], F32, name="xrc", tag="xrc")
                nc.vector.tensor_mul(xrc[:], s0c[:], s0c[:])
                nc.vector.scalar_tensor_tensor(
                    xrc[:], xrc[:], -1.0 / (E * 4096.0), st_q[:], op0=OP.mult, op1=OP.add
                )
                nc.scalar.activation(
                    rB[:, rsl], xrc[:], AF.Abs_reciprocal_sqrt,
                    bias=eps1[:], scale=4096.0 / E,
                )
                nc.vector.scalar_tensor_tensor(
                    rB[:, msl], s0c[:], 1.0 / E, rB[:, rsl], op0=OP.mult, op1=OP.mult,
                )

            def apply_steps(b, Y, g_col, nbe_col):
                """Filler closures for the LN apply (1024-wide ops)."""
                rB = rowsB[b]
                steps = []
                bc_box = [None, None]

                def mk_bcast(c2):
                    def f():
                        bc = bcpool.tile([P, 2048], F16, name="bc", tag="bc")
                        nc.gpsimd.partition_broadcast(
                            bc[:], rB[:, c2 * 2048 : (c2 + 1) * 2048]
                        )
                        bc_box[c2] = bc
                    return f

                def mk_apply(c2, pt):
                    def f():
                        bc = bc_box[c2]
                        cs2 = slice(c2 * 1024, (c2 + 1) * 1024)
                        rbv = bc[:, 0:1024]
                        mbv = bc[:, 1024:2048]
                        mg = mgpool.tile([P, 1024], F16, name="mg", tag="mg")
                        nc.scalar.activation(
                            mg[:], mbv, AF.Identity,
                            scale=g_col[:, pt : pt + 1], bias=nbe_col[:, pt : pt + 1],
                        )
                        t1 = tpool.tile([P, 1024], F16, name="t1", tag="t1")
                        nc.vector.scalar_tensor_tensor(
                            t1[:], Y[pt][:, cs2], g_col[:, pt : pt + 1], rbv,
                            op0=OP.mult, op1=OP.mult,
                        )
                        nc.vector.tensor_sub(Y[pt][:, cs2], t1[:], mg[:])
                    return f

                for c2 in range(CH // 2):
                    steps.append(mk_bcast(c2))
                    for pt in range(EC):
                        steps.append(mk_apply(c2, pt))
                return steps

            def apply_final_steps(b, Y):
                """Fillers for the last LN2 + readout product accumulation.
                DVE-only; collects pr tiles, ro matmuls emitted separately."""
                rB = rowsB[b]
                pr_box = []
                steps = []
                bc_box = [None, None]

                def mk_bcast(c2):
                    def f():
                        bc = bcpool.tile([P, 2048], F16, name="bc", tag="bc")
                        nc.gpsimd.partition_broadcast(
                            bc[:], rB[:, c2 * 2048 : (c2 + 1) * 2048]
                        )
                        bc_box[c2] = bc
                    return f

                def mk_apply(c2, pt):
                    def f():
                        bc = bc_box[c2]
                        cs2 = slice(c2 * 1024, (c2 + 1) * 1024)
                        rbv = bc[:, 0:1024]
                        mbv = bc[:, 1024:2048]
                        t1 = tpool.tile([P, 1024], F16, name="t1", tag="t1")
                        nc.vector.tensor_mul(t1[:], Y[pt][:, cs2], rbv)
                        t2 = tpool.tile([P, 1024], F16, name="t2", tag="t2")
                        nc.vector.tensor_sub(t2[:], t1[:], mbv)
                        pr = prpool.tile([P, 1024], F16, name="pr", tag="pr")
                        nc.vector.tensor_mul(pr[:], t2[:], gw_sb[pt][:, cs2])
                        pr_box.append(pr)
                    return f

                for c2 in range(CH // 2):
                    steps.append(mk_bcast(c2))
                    for pt in range(EC):
                        steps.append(mk_apply(c2, pt))
                return steps, pr_box

            def readout_tail(b, pr_box):
                ro = ps_o.tile([1, 512], F32, name="rops", tag="o")
                nmm = 0
                for pr in pr_box:
                    for half in range(2):
                        nc.tensor.matmul(
                            ro[:], ones_kb[:], pr[:, half * 512 : (half + 1) * 512],
                            start=(nmm == 0), stop=(nmm == 2 * len(pr_box) - 1),
                        )
                        nmm += 1
                rsum = ropool.tile([1, 1], F32, name="rsum", tag="rsum")
                nc.vector.reduce_sum(rsum[:], ro[:], axis=mybir.AxisListType.X)
                ob = ropool.tile([1, 1], F32, name="ob", tag="ob")
                nc.scalar.activation(ob[:], rsum[:], AF.Identity, bias=bout_sb[:])
                nc.sync.dma_start(d_out[b : b + 1, :], ob[:])

            # ---- attention / mlp with integrated stats pipeline ---------
            def attention(b, X, Y):
                for c in range(CH):
                    cs = slice(c * 512, (c + 1) * 512)
                    o_ps = [
                        ps_o.tile([P, 512], F32, name=f"o{oc}", tag="o")
                        for oc in range(EC)
                    ]

                    def emit_S(j2):
                        s_ps = ps_s.tile([P, 1024], F32, name="s_ps", tag="s")
                        for h in range(2):
                            j = 2 * j2 + h
                            hs = slice(h * 512, (h + 1) * 512)
                            for dc in range(EC):
                                nc.tensor.matmul(
                                    s_ps[:, hs],
                                    X[dc][:, j * P : (j + 1) * P],
                                    kT[b][dc][:, cs],
                                    start=(dc == 0),
                                    stop=(dc == EC - 1),
                                )
                        return s_ps

                    s_prev = emit_S(0)
                    for j2 in range(JT // 2):
                        sr = spool.tile([P, 1024], F16, name="sr", tag="sr")
                        if j2 % 2 == 0:
                            nc.scalar.activation(sr[:], s_prev[:], AF.Relu)
                        else:
                            nc.vector.tensor_relu(sr[:], s_prev[:])
                        s_next = emit_S(j2 + 1) if j2 + 1 < JT // 2 else None
                        for h in range(2):
                            j = 2 * j2 + h
                            hs = slice(h * 512, (h + 1) * 512)
                            for oc in range(EC):
                                nc.tensor.matmul(
                                    o_ps[oc][:],
                                    v_sb[b][:, j * E + oc * P : j * E + (oc + 1) * P],
                                    sr[:, hs],
                                    start=(j == 0),
                                    stop=(j == JT - 1),
                                )
                        s_prev = s_next
                    for oc in range(EC):
                        nc.vector.tensor_add(Y[oc][:, cs], X[oc][:, cs], o_ps[oc][:])
                    if c % 2 == 1:
                        stats_sq(b, Y, c // 2)
                    if c >= 1:
                        stats_mm(b, Y, c - 1)
                    pop_fill(2)

            def mlp(b, l, Y):
                w1 = w_sb["W1"][l]
                w2 = w_sb["W2"][l]

                def emit_w1(c):
                    cs = slice(c * 512, (c + 1) * 512)
                    ats = []
                    for mc in range(EC):
                        ms = slice(mc * P, (mc + 1) * P)
                        ps = ps_s.tile([P, 512], F32, name="psa", tag="s")
                        for ec in range(EC):
                            nc.tensor.matmul(
                                ps[:], w1[ec][:, ms], Y[ec][:, cs],
                                start=(ec == 0), stop=(ec == EC - 1),
                            )
                        a = apool.tile([P, 512], F16, name="a", tag="a")
                        nc.scalar.activation(
                            a[:], ps[:], AF.Relu, bias=bm1_sb[l][:, mc : mc + 1]
                        )
                        ats.append(a)
                    return ats

                def emit_w2(c, ats):
                    cs = slice(c * 512, (c + 1) * 512)
                    for oc in range(EC):
                        os_ = slice(oc * P, (oc + 1) * P)
                        ps = ps_o.tile([P, 512], F32, name="psm", tag="o")
                        for mc in range(EC):
                            nc.tensor.matmul(
                                ps[:], w2[mc][:, os_], ats[mc][:],
                                start=(mc == 0), stop=(mc == EC - 1),
                            )
                        nc.vector.scalar_tensor_tensor(
                            Y[oc][:, cs], ps[:], bm2_sb[l][:, oc : oc + 1],
                            Y[oc][:, cs], op0=OP.add, op1=OP.add,
                        )

                prev = emit_w1(0)
                for c in range(CH):
                    nxt = emit_w1(c + 1) if c + 1 < CH else None
                    emit_w2(c, prev)
                    prev = nxt
                    if c % 2 == 1:
                        stats_sq(b, Y, c // 2)
                    if c >= 1:
                        stats_mm(b, Y, c - 1)
                    pop_fill(2)

            # ---- interleaved schedule -----------------------------------
            cur, alt = HA, HB
            input_proj(0, cur[0])
            input_proj(1, cur[1])
            for l in range(L):
                X0, Y0 = cur[0], alt[0]
                X1, Y1 = cur[1], alt[1]
                kv(0, l, X0)
                kv(1, l, X1)
                flush_fill()
                attention(0, X0, Y0)
                q.append(lambda b=0, Y=Y0: stats_mm(b, Y, CH - 1))
                q.extend(apply_steps(0, Y0, g1_sb[l], nbe1_sb[l]))
                attention(1, X1, Y1)
                q.append(lambda b=1, Y=Y1: stats_mm(b, Y, CH - 1))
                q.extend(apply_steps(1, Y1, g1_sb[l], nbe1_sb[l]))
                mlp(0, l, Y0)
                q.append(lambda b=0, Y=Y0: stats_mm(b, Y, CH - 1))
                if l < L - 1:
                    q.extend(apply_steps(0, Y0, g2_sb[l], nbe2_sb[l]))
                    mlp(1, l, Y1)
                    q.append(lambda b=1, Y=Y1: stats_mm(b, Y, CH - 1))
                    q.extend(apply_steps(1, Y1, g2_sb[l], nbe2_sb[l]))
                else:
                    steps0, pr0 = apply_final_steps(0, Y0)
                    q.extend(steps0)
                    mlp(1, l, Y1)
                    q.append(lambda b=1, Y=Y1: stats_mm(b, Y, CH - 1))
                    flush_fill()
                    readout_tail(0, pr0)
                    steps1, pr1 = apply_final_steps(1, Y1)
                    for s in steps1:
                        s()
                    readout_tail(1, pr1)
                cur, alt = alt, cur

    nc.compile()
    return nc


def _prep_inputs(inputs):
    f = lambda x: np.asarray(x, np.float32)
    bf = lambda x: np.ascontiguousarray(np.asarray(x, np.float32).astype(NPF16))
    xs = f(inputs["xs"])
    xsT = np.ascontiguousarray(xs.transpose(0, 2, 1)).astype(NPF16)  # [B, D, N]
    WoutT = np.ascontiguousarray(f(inputs["Wout"]).reshape(N, E).T)  # [E, N]
    G = np.einsum("lij,lkj->lik", f(inputs["Wq"]), f(inputs["Wk"]))  # Wq @ Wk^T
    g2_last = f(inputs["g2"])[L - 1]  # [E]
    be2_last = f(inputs["be2"])[L - 1]
    gwoutT = (g2_last[:, None] * WoutT).astype(NPF16)  # [E, N]
    bout_adj = f(inputs["b_out"]).reshape(1, 1) + np.sum(be2_last[:, None] * WoutT)

    def cols(v, per_l):
        v = f(v)
        if per_l:
            return np.ascontiguousarray(v.reshape(L, EC, P).transpose(0, 2, 1))
        return np.ascontiguousarray(v.reshape(EC, P).T)

    common = {
        "Win": bf(inputs["Win"]),
        "G": G.astype(NPF16),
        "Wv": bf(inputs["Wv"]),
        "W1": bf(inputs["W1"]),
        "W2": bf(inputs["W2"]),
        "gwoutT": gwoutT,
        "colpack": np.concatenate(
            [cols(inputs["b_in"], False)]
            + [
                cols(inputs[k], True).transpose(1, 0, 2).reshape(P, L * EC)
                for k in ("bm1", "bm2", "be1", "be2", "g1", "g2")
            ]
            + [
                cols(-np.asarray(inputs[k], np.float32), True)
                .transpose(1, 0, 2)
                .reshape(P, L * EC)
                for k in ("be1", "be2")
            ],
            axis=1,
        ),
        "b_out": bout_adj.astype(np.float32),
    }
    in_maps = []
    for c in range(NCORES):
        m = dict(common)
        m["xsT"] = np.ascontiguousarray(xsT[c * BL : (c + 1) * BL])
        in_maps.append(m)
    return in_maps


def get_program():
    if "nc" not in _CACHE:
        _CACHE["nc"] = _build()
    return _CACHE["nc"]


def kernel(**inputs) -> np.ndarray:
    nc = get_program()
    in_maps = _prep_inputs(inputs)
    res = run_bass_kernel_spmd(nc, in_maps, list(range(NCORES)))
    out = np.concatenate([res.results[c]["out"] for c in range(NCORES)], axis=0)
    return out.astype(np.float32)
